# revision 8
# baseline (speedup 1.0000x reference)
"""GCN2 (GCNII) forward pass on 8 Trainium2 NeuronCores (Bass/Tile SPMD).

Strategy (matches the sharding hint):
 - 50000 nodes are packed into 8 cores x 196 windows x 32 slots (host-side
   bin packing balances in-degree so every window holds <= 512 edges).
 - Per layer, each core aggregates messages for its own 6272 node slots:
   a dma_gather pulls source-node features (fp16 rows, 256B) from the
   replicated h-table in HBM, and one-hot S matmuls on the tensor engine
   compute the segment sums straight into PSUM (agg^T layout [96, dst]).
 - The initial-residual x0 term is added via a diagonal matmul; the conv
   weight, (1-beta)/beta blend, 0.9 edge weight and a per-layer 1/13.5
   rescaling (keeps activations in fp16 range) are folded into the weights
   host-side.
 - After each layer an HBM AllGather replicates the new shard into every
   core's h-table. Small weights are replicated.
 - Layer 0 (x @ W_in) runs in fp32; the output head applies
   log_softmax over the 64 classes on-chip.
"""
import sys
for _p in ('/opt/trn_rl_repo', '/root/.axon_site/_ro/trn_rl_repo'):
    if _p not in sys.path:
        sys.path.insert(0, _p)
import math
import numpy as np

N = 50000
E = 800000
NFEAT = 512
HID = 96
NCLASS = 64
NUM_LAYERS = 8
ALPHA, THETA = 0.1, 0.5

N_CORES = 8
P = 128
W = 32                     # dst slots per window
CW = 4                     # chunks (of 128 edges) per window: 2 A + 2 B
NWIN = 196
NODES = NWIN * W           # 6272 node slots per core
CHUNKS = NWIN * CW         # 784
NTAB = N_CORES * NODES     # 50176 table rows
G = 13.5                   # per-layer growth folded out of the activations
NBINS = N_CORES * NWIN
BANK_W = 16                # windows per PSUM bank tile
NBANKS = 13
A_LIMIT = 32768
B_BASE = NTAB - 32768      # 17408
GRP = 2 * P                # 256 edge slots per (window, table-half) group
KBLK = NFEAT // P
NBLK = NODES // P          # 49


# ---------------------------------------------------------------------------
# host-side preprocessing
# ---------------------------------------------------------------------------

def _pack_nodes(dst):
    """Assign each node to a (core, window, slot); balance in-degree."""
    deg = np.bincount(dst, minlength=N).astype(np.int64)
    order = np.argsort(-deg, kind="stable")
    cap_e = CW * P
    import heapq
    heap = [(0, b) for b in range(NBINS)]
    heapq.heapify(heap)
    bin_edges = np.zeros(NBINS, np.int64)
    bin_slots = np.zeros(NBINS, np.int64)
    assign = np.empty(N, np.int64)
    slot_of = np.empty(N, np.int64)
    for v in order:
        d = deg[v]
        popped = []
        while True:
            key, b = heapq.heappop(heap)
            if bin_slots[b] < W and bin_edges[b] + d <= cap_e:
                break
            popped.append((key, b))
        for it in popped:
            heapq.heappush(heap, it)
        assign[v] = b
        slot_of[v] = bin_slots[b]
        bin_slots[b] += 1
        bin_edges[b] += d
        if bin_slots[b] < W:
            heapq.heappush(heap, (int(bin_edges[b] + 16 * bin_slots[b]), b))
    assert bin_edges.max() <= cap_e and bin_slots.max() <= W
    return assign * W + slot_of, deg


def _host_prep(inputs):
    x = np.asarray(inputs["x"], np.float32)
    edge_index = np.asarray(inputs["edge_index"])
    W_in = np.asarray(inputs["W_in"], np.float32)
    b_in = np.asarray(inputs["b_in"], np.float32)
    conv_W = np.asarray(inputs["conv_W"], np.float32)
    W_out = np.asarray(inputs["W_out"], np.float32)
    b_out = np.asarray(inputs["b_out"], np.float32)
    src, dst = edge_index[0].astype(np.int64), edge_index[1].astype(np.int64)

    pos, _deg = _pack_nodes(dst)

    ebin = pos[dst] // W
    erel = (pos[dst] % W).astype(np.int64)
    erow = pos[src]
    eclass = np.where(erow < B_BASE, 0, np.where(erow >= A_LIMIT, 2, 1))
    eorder = np.argsort(ebin * 4 + eclass, kind="stable")
    ebin_s, erel_s = ebin[eorder], erel[eorder]
    erow_s, ecls_s = erow[eorder], eclass[eorder]

    bin_tot = np.bincount(ebin_s, minlength=NBINS)
    bin_a0 = np.bincount(ebin_s[ecls_s == 0], minlength=NBINS)
    bin_b0 = np.bincount(ebin_s[ecls_s == 2], minlength=NBINS)
    assert bin_a0.max() <= GRP and bin_b0.max() <= GRP
    nA = np.maximum(np.minimum(GRP, bin_tot - bin_b0), bin_tot - GRP)
    assert (nA >= bin_a0).all() and (bin_tot - nA <= GRP).all()

    bin_starts = np.zeros(NBINS + 1, np.int64)
    np.cumsum(bin_tot, out=bin_starts[1:])
    within = np.arange(E) - bin_starts[ebin_s]
    in_A = within < nA[ebin_s]
    gslot = np.where(in_A, within, within - nA[ebin_s])

    rowA = np.zeros((NBINS, GRP), np.int64)
    relA = np.full((NBINS, GRP), -1, np.int64)
    rowB = np.full((NBINS, GRP), B_BASE, np.int64)
    relB = np.full((NBINS, GRP), -1, np.int64)
    rowA[ebin_s[in_A], gslot[in_A]] = erow_s[in_A]
    relA[ebin_s[in_A], gslot[in_A]] = erel_s[in_A]
    rowB[ebin_s[~in_A], gslot[~in_A]] = erow_s[~in_A]
    relB[ebin_s[~in_A], gslot[~in_A]] = erel_s[~in_A]
    assert rowA.max() < A_LIMIT and rowB.min() >= B_BASE

    idxA_c = rowA.astype(np.int16).reshape(N_CORES, NWIN * GRP)
    idxB_c = (rowB - B_BASE).astype(np.int16).reshape(N_CORES, NWIN * GRP)

    def wrap(a):
        w16 = a.reshape(N_CORES, NWIN * GRP // 16, 16).transpose(0, 2, 1)
        return np.ascontiguousarray(np.tile(w16, (1, 8, 1)))

    idxA_w, idxB_w = wrap(idxA_c), wrap(idxB_c)

    S = np.zeros((N_CORES, P, CHUNKS, W), np.float16)

    def fill_S(rel, is_B):
        binid, slot = np.nonzero(rel >= 0)
        r = rel[binid, slot]
        core, win = binid // NWIN, binid % NWIN
        bank, w_local = win // BANK_W, win % BANK_W
        nwin_b = np.where(bank < NBANKS - 1, BANK_W, NWIN - BANK_W * (NBANKS - 1))
        sub = w_local * 2 + slot // P + (nwin_b * 2 if is_B else 0)
        S[core, slot % P, bank * 64 + sub, r] = np.float16(1.0)

    fill_S(relA, False)
    fill_S(relB, True)

    nodes_at = np.full((N_CORES, NODES), -1, np.int64)
    nodes_at[pos // NODES, pos % NODES] = np.arange(N)
    xT = np.zeros((N_CORES, P, KBLK, NODES), np.float32)
    for c in range(N_CORES):
        ids = nodes_at[c]
        valid = ids >= 0
        Xc = np.zeros((NODES, NFEAT), np.float32)
        Xc[valid] = x[ids[valid]]
        xT[c] = Xc.T.reshape(KBLK, P, NODES).transpose(1, 0, 2)

    W_in_r = np.ascontiguousarray(
        W_in.reshape(KBLK, P, HID).transpose(1, 0, 2)).astype(np.float32)
    b_in_rep = np.tile(b_in[None, :], (P, 4)).astype(np.float32)
    b_out_rep = np.tile(b_out[None, :], (P, 1)).astype(np.float32)
    Wc_hat = np.zeros((NUM_LAYERS, HID, HID), np.float16)
    diag = np.zeros((NUM_LAYERS, HID, HID), np.float32)
    for l in range(NUM_LAYERS):
        beta = math.log(THETA / (l + 1) + 1.0)
        Wt = (1.0 - beta) * np.eye(HID, dtype=np.float32) + beta * conv_W[l]
        Wc_hat[l] = (0.9 / G * Wt).astype(np.float16)
        diag[l] = np.eye(HID, dtype=np.float32) * (G ** (-float(l)))
    Wc_sb = np.ascontiguousarray(Wc_hat.transpose(1, 0, 2))
    diag_sb = np.ascontiguousarray(diag.transpose(1, 0, 2))

    per_core = []
    for c in range(N_CORES):
        per_core.append({
            "xT": xT[c], "idxA": idxA_w[c], "idxB": idxB_w[c], "S": S[c],
            "W_in": W_in_r, "b_in": b_in_rep, "Wc": Wc_sb, "diag": diag_sb,
            "W_out": W_out.astype(np.float16), "b_out": b_out_rep,
        })
    return per_core, nodes_at


# ---------------------------------------------------------------------------
# bass program
# ---------------------------------------------------------------------------

def _nwin_of_bank(b):
    return BANK_W if b < NBANKS - 1 else NWIN - BANK_W * (NBANKS - 1)


def build_program(repeat=1, skip_collective=False, skip_gather=False):
    import concourse.bacc as bacc
    import concourse.tile as tile
    import concourse.mybir as mybir
    from concourse.masks import make_identity

    F32, F16, I16 = mybir.dt.float32, mybir.dt.float16, mybir.dt.int16
    nc = bacc.Bacc("TRN2", target_bir_lowering=False, debug=False,
                   num_devices=N_CORES)

    xT = nc.dram_tensor("xT", [P, KBLK, NODES], F32, kind="ExternalInput")
    idxA = nc.dram_tensor("idxA", [P, NWIN * GRP // 16], I16, kind="ExternalInput")
    idxB = nc.dram_tensor("idxB", [P, NWIN * GRP // 16], I16, kind="ExternalInput")
    S_in = nc.dram_tensor("S", [P, CHUNKS, W], F16, kind="ExternalInput")
    Wi_in = nc.dram_tensor("W_in", [P, KBLK, HID], F32, kind="ExternalInput")
    bi_in = nc.dram_tensor("b_in", [P, 4 * HID], F32, kind="ExternalInput")
    Wc_in = nc.dram_tensor("Wc", [HID, NUM_LAYERS, HID], F16, kind="ExternalInput")
    diag_in = nc.dram_tensor("diag", [HID, NUM_LAYERS, HID], F32, kind="ExternalInput")
    Wo_in = nc.dram_tensor("W_out", [HID, NCLASS], F16, kind="ExternalInput")
    bo_in = nc.dram_tensor("b_out", [P, NCLASS], F32, kind="ExternalInput")
    out_d = nc.dram_tensor("out", [NODES, NCLASS], F32, kind="ExternalOutput")

    with tile.TileContext(nc) as tc:
        with tc.tile_pool(name="dram", bufs=1, space="DRAM") as dram, \
             tc.tile_pool(name="res", bufs=1) as res, \
             tc.tile_pool(name="mpool", bufs=3) as mpool, \
             tc.tile_pool(name="psA", bufs=2, space="PSUM") as psA, \
             tc.tile_pool(name="psB", bufs=2, space="PSUM") as psB, \
             tc.tile_pool(name="psT", bufs=2, space="PSUM") as psT:

            table0 = dram.tile([NTAB, P], F16, name="table0")
            table1 = dram.tile([NTAB, P], F16, name="table1")
            shard_d = dram.tile([NODES, P], F16, name="shard")
            tables = [table0, table1]

            S_sb = res.tile([P, CHUNKS, W], F16)
            nc.sync.dma_start(S_sb[:], S_in[:])
            idxA_sb = res.tile([P, NWIN * GRP // 16], I16)
            nc.sync.dma_start(idxA_sb[:], idxA[:])
            idxB_sb = res.tile([P, NWIN * GRP // 16], I16)
            nc.sync.dma_start(idxB_sb[:], idxB[:])
            Wi_sb = res.tile([P, KBLK, HID], F32)
            nc.sync.dma_start(Wi_sb[:], Wi_in[:])
            bi_sb = res.tile([P, 4 * HID], F32)
            nc.sync.dma_start(bi_sb[:], bi_in[:])
            Wc_sb = res.tile([HID, NUM_LAYERS, HID], F16)
            nc.sync.dma_start(Wc_sb[:], Wc_in[:])
            diag_sb = res.tile([HID, NUM_LAYERS, HID], F32)
            nc.sync.dma_start(diag_sb[:], diag_in[:])
            Wo_sb = res.tile([HID, NCLASS], F16)
            nc.sync.dma_start(Wo_sb[:], Wo_in[:])
            bo_sb = res.tile([P, NCLASS], F32)
            nc.sync.dma_start(bo_sb[:], bo_in[:])
            ident = res.tile([P, P], F16)
            make_identity(nc, ident[:])

            hstage = res.tile([P, NBLK, P], F16)
            nc.vector.memset(hstage[:], 0.0)
            x0t = res.tile([HID, NODES], F32)
            hc = res.tile([HID, NODES], F16)
            h8T = res.tile([HID, NODES], F16)
            logits = res.tile([P, NBLK, NCLASS], F32)
            et = res.tile([P, NBLK, NCLASS], F32)
            mx = res.tile([P, NBLK], F32)
            sm = res.tile([P, NBLK], F32)
            lns = res.tile([P, NBLK], F32)

            for _rep in range(repeat):
                # -------- layer 0: h0 = relu(x @ W_in + b_in) --------
                for q in range(NBANKS):
                    nbl = 4 if q < NBANKS - 1 else 1
                    xt = mpool.tile([P, KBLK, 512], F32, tag="m", name="xt")
                    nc.sync.dma_start(xt[:, :, :nbl * P],
                                      xT[:, :, q * 512: q * 512 + nbl * P])
                    ps0 = psB.tile([P, 4 * HID], F32, tag="psB", name="ps0")
                    for t in range(nbl):
                        for k in range(KBLK):
                            nc.tensor.matmul(
                                ps0[:, t * HID:(t + 1) * HID],
                                lhsT=xt[:, k, t * P:(t + 1) * P],
                                rhs=Wi_sb[:, k, :],
                                start=(k == 0), stop=(k == KBLK - 1))
                    nc.vector.tensor_add(ps0[:, :nbl * HID], ps0[:, :nbl * HID],
                                         bi_sb[:, :nbl * HID])
                    nc.scalar.activation(
                        hstage[:, q * 4: q * 4 + nbl, :HID],
                        ps0[:, :nbl * HID],
                        mybir.ActivationFunctionType.Relu)
                for blk in range(NBLK):
                    pst = psT.tile([HID, P], F16, tag="psT", name="pst")
                    nc.tensor.transpose(pst[:], hstage[:, blk, :HID], ident[:])
                    nc.vector.tensor_scalar_mul(
                        x0t[:, blk * P:(blk + 1) * P], pst[:], 1.0 / 9.0)
                nc.sync.dma_start(
                    shard_d[:].rearrange("(b p) f -> p b f", p=P), hstage[:])
                if not skip_collective:
                    nc.gpsimd.collective_compute(
                        "AllGather", mybir.AluOpType.bypass,
                        replica_groups=[list(range(N_CORES))],
                        ins=[shard_d.opt()], outs=[tables[0].opt()])
                else:
                    nc.sync.dma_start(tables[0][:NODES, :],
                                      shard_d[:])

                # -------- propagation layers -------------------------
                for layer in range(NUM_LAYERS):
                    tab = tables[layer % 2]
                    tab_next = tables[(layer + 1) % 2]
                    for b in range(NBANKS):
                        nw = _nwin_of_bank(b)
                        nchA = nw * 2
                        cols = nw * W
                        m = mpool.tile([P, 64, P], F16, tag="m", name="m")
                        if skip_gather:
                            # contiguous pseudo-gather: same bytes, sequential
                            nc.sync.dma_start(
                                m[:, :nchA * 2, :],
                                tab[:nchA * 2 * P, :]
                                .rearrange("(c p) f -> p c f", p=P))
                            nc.gpsimd.dma_gather(
                                out_ap=m[:, :nchA, :],
                                in_ap=tab[:A_LIMIT, :],
                                idxs_ap=idxA_sb[:, b * 256: b * 256 + nchA * 8],
                                num_idxs=nchA * P, num_idxs_reg=nchA * P,
                                elem_size=P, single_packet=False)
                            nc.gpsimd.dma_gather(
                                out_ap=m[:, nchA:nchA * 2, :],
                                in_ap=tab[B_BASE:, :],
                                idxs_ap=idxB_sb[:, b * 256: b * 256 + nchA * 8],
                                num_idxs=nchA * P, num_idxs_reg=nchA * P,
                                elem_size=P, single_packet=False)
                        pA = psA.tile([P, BANK_W * W], F32, tag="psA", name="pA")
                        nc.tensor.matmul(
                            pA[:HID, :cols],
                            lhsT=diag_sb[:, layer, :],
                            rhs=x0t[:, b * 512: b * 512 + cols],
                            start=True, stop=False, skip_group_check=True)
                        for w in range(nw):
                            col = w * W
                            for t, sub in enumerate(
                                    (w * 2, w * 2 + 1,
                                     nchA + w * 2, nchA + w * 2 + 1)):
                                nc.tensor.matmul(
                                    pA[:HID, col:col + W],
                                    lhsT=m[:, sub, :HID],
                                    rhs=S_sb[:, b * 64 + sub, :],
                                    start=False,
                                    stop=(w == nw - 1 and t == 3),
                                    skip_group_check=True)
                        nc.vector.tensor_copy(
                            hc[:, b * 512: b * 512 + cols], pA[:HID, :cols])
                        if layer < NUM_LAYERS - 1:
                            ps1 = psB.tile([P, 4 * HID], F32, tag="psB", name="ps1")
                            nbl = cols // P
                            for t in range(nbl):
                                nc.tensor.matmul(
                                    ps1[:, t * HID:(t + 1) * HID],
                                    lhsT=hc[:, b * 512 + t * P:
                                            b * 512 + (t + 1) * P],
                                    rhs=Wc_sb[:, layer, :],
                                    start=True, stop=True)
                            nc.scalar.activation(
                                hstage[:, b * 4: b * 4 + nbl, :HID],
                                ps1[:, :nbl * HID],
                                mybir.ActivationFunctionType.Relu)
                        else:
                            pC = psA.tile([P, BANK_W * W], F32, tag="psA", name="pC")
                            nc.tensor.matmul(
                                pC[:HID, :cols],
                                lhsT=Wc_sb[:, layer, :],
                                rhs=hc[:, b * 512: b * 512 + cols],
                                start=True, stop=True)
                            nc.scalar.activation(
                                h8T[:, b * 512: b * 512 + cols],
                                pC[:HID, :cols],
                                mybir.ActivationFunctionType.Relu)
                    if layer < NUM_LAYERS - 1:
                        nc.sync.dma_start(
                            shard_d[:].rearrange("(b p) f -> p b f", p=P),
                            hstage[:])
                        if not skip_collective:
                            nc.gpsimd.collective_compute(
                                "AllGather", mybir.AluOpType.bypass,
                                replica_groups=[list(range(N_CORES))],
                                ins=[shard_d.opt()], outs=[tab_next.opt()])
                        else:
                            nc.sync.dma_start(tab_next[:NODES, :], shard_d[:])

                # -------- output head --------------------------------
                for blk in range(NBLK):
                    psD = psB.tile([P, 4 * HID], F32, tag="psB", name="psD")
                    nc.tensor.matmul(
                        psD[:, :NCLASS],
                        lhsT=h8T[:, blk * P:(blk + 1) * P],
                        rhs=Wo_sb[:],
                        start=True, stop=True)
                    nc.vector.tensor_scalar_mul(
                        logits[:, blk, :], psD[:, :NCLASS],
                        float(G ** NUM_LAYERS))
                nc.vector.tensor_add(
                    logits[:], logits[:],
                    bo_sb[:, None, :].to_broadcast([P, NBLK, NCLASS]))
                nc.vector.tensor_reduce(mx[:], logits[:],
                                        axis=mybir.AxisListType.X,
                                        op=mybir.AluOpType.max)
                nc.vector.tensor_sub(
                    logits[:], logits[:],
                    mx[:, :, None].to_broadcast([P, NBLK, NCLASS]))
                nc.scalar.activation(et[:], logits[:],
                                     mybir.ActivationFunctionType.Exp)
                nc.vector.tensor_reduce(sm[:], et[:],
                                        axis=mybir.AxisListType.X,
                                        op=mybir.AluOpType.add)
                nc.scalar.activation(lns[:], sm[:],
                                     mybir.ActivationFunctionType.Ln)
                nc.vector.tensor_sub(
                    logits[:], logits[:],
                    lns[:, :, None].to_broadcast([P, NBLK, NCLASS]))
                nc.sync.dma_start(
                    out_d[:].rearrange("(b p) f -> p b f", p=P), logits[:])

    nc.compile()
    return nc


# ---------------------------------------------------------------------------
# entry point
# ---------------------------------------------------------------------------

_CACHED_NC = None


def kernel(**inputs):
    global _CACHED_NC
    from concourse.bass_utils import run_bass_kernel_spmd

    per_core, nodes_at = _host_prep(inputs)
    if _CACHED_NC is None:
        _CACHED_NC = build_program(repeat=1)
    nc = _CACHED_NC
    res = run_bass_kernel_spmd(nc, per_core, core_ids=list(range(N_CORES)))
    out = np.zeros((N, NCLASS), np.float32)
    for c in range(N_CORES):
        ids = nodes_at[c]
        valid = ids >= 0
        out[ids[valid]] = res.results[c]["out"][valid]
    return out


# revision 14
# speedup vs baseline: 3.4957x; 3.4957x over previous
"""GCN2 (GCNII) forward pass on 8 Trainium2 NeuronCores (Bass/Tile SPMD).

Strategy (matches the sharding hint):
 - 50000 nodes are packed into 8 cores x 196 windows x 32 slots (host-side
   bin packing balances in-degree so every window holds <= 512 edges).
 - Per layer, each core aggregates messages for its own 6272 node slots:
   a dma_gather pulls source-node features (fp16 rows, 256B) from the
   replicated h-table in HBM, and one-hot S matmuls on the tensor engine
   compute the segment sums straight into PSUM (agg^T layout [96, dst]).
 - The initial-residual x0 term is added via a diagonal matmul; the conv
   weight, (1-beta)/beta blend, 0.9 edge weight and a per-layer 1/13.5
   rescaling (keeps activations in fp16 range) are folded into the weights
   host-side.
 - After each layer an HBM AllGather replicates the new shard into every
   core's h-table. Small weights are replicated.
 - Layer 0 (x @ W_in) runs in fp32; the output head applies
   log_softmax over the 64 classes on-chip.
"""
import sys
for _p in ('/opt/trn_rl_repo', '/root/.axon_site/_ro/trn_rl_repo'):
    if _p not in sys.path:
        sys.path.insert(0, _p)
import math
import numpy as np

N = 50000
E = 800000
NFEAT = 512
HID = 96
NCLASS = 64
NUM_LAYERS = 8
ALPHA, THETA = 0.1, 0.5

N_CORES = 8
P = 128
W = 32                     # dst slots per window
CW = 4                     # chunks (of 128 edges) per window: 2 A + 2 B
NWIN = 196
NODES = NWIN * W           # 6272 node slots per core
CHUNKS = NWIN * CW         # 784
NTAB = N_CORES * NODES     # 50176 table rows
G = 13.5                   # per-layer growth folded out of the activations
NBINS = N_CORES * NWIN
BANK_W = 16                # windows per PSUM bank tile
NBANKS = 13
A_LIMIT = 32768
B_BASE = NTAB - 32768      # 17408
GRP = 2 * P                # 256 edge slots per (window, table-half) group
KBLK = NFEAT // P
NBLK = NODES // P          # 49


# ---------------------------------------------------------------------------
# host-side preprocessing
# ---------------------------------------------------------------------------

def _pack_nodes(dst):
    """Assign each node to a (core, window, slot); balance in-degree."""
    deg = np.bincount(dst, minlength=N).astype(np.int64)
    order = np.argsort(-deg, kind="stable")
    cap_e = CW * P
    assert deg.max() <= cap_e, f"node in-degree {deg.max()} exceeds window capacity" 
    import heapq
    heap = [(0, b) for b in range(NBINS)]
    heapq.heapify(heap)
    bin_edges = np.zeros(NBINS, np.int64)
    bin_slots = np.zeros(NBINS, np.int64)
    assign = np.empty(N, np.int64)
    slot_of = np.empty(N, np.int64)
    for v in order:
        d = deg[v]
        popped = []
        while True:
            key, b = heapq.heappop(heap)
            if bin_slots[b] < W and bin_edges[b] + d <= cap_e:
                break
            popped.append((key, b))
        for it in popped:
            heapq.heappush(heap, it)
        assign[v] = b
        slot_of[v] = bin_slots[b]
        bin_slots[b] += 1
        bin_edges[b] += d
        if bin_slots[b] < W:
            heapq.heappush(heap, (int(bin_edges[b] + 16 * bin_slots[b]), b))
    assert bin_edges.max() <= cap_e and bin_slots.max() <= W
    return assign * W + slot_of, deg


def _host_prep(inputs):
    x = np.asarray(inputs["x"], np.float32)
    edge_index = np.asarray(inputs["edge_index"])
    W_in = np.asarray(inputs["W_in"], np.float32)
    b_in = np.asarray(inputs["b_in"], np.float32)
    conv_W = np.asarray(inputs["conv_W"], np.float32)
    W_out = np.asarray(inputs["W_out"], np.float32)
    b_out = np.asarray(inputs["b_out"], np.float32)
    src, dst = edge_index[0].astype(np.int64), edge_index[1].astype(np.int64)

    pos, _deg = _pack_nodes(dst)

    ebin = pos[dst] // W
    erel = (pos[dst] % W).astype(np.int64)
    erow = pos[src]
    eclass = np.where(erow < B_BASE, 0, np.where(erow >= A_LIMIT, 2, 1))
    # sort by (bin, class, src-row): rows ascend within each gather group,
    # improving HBM locality of the random gather
    eorder = np.lexsort((erow, ebin * 4 + eclass))
    ebin_s, erel_s = ebin[eorder], erel[eorder]
    erow_s, ecls_s = erow[eorder], eclass[eorder]

    bin_tot = np.bincount(ebin_s, minlength=NBINS)
    bin_a0 = np.bincount(ebin_s[ecls_s == 0], minlength=NBINS)
    bin_b0 = np.bincount(ebin_s[ecls_s == 2], minlength=NBINS)
    assert bin_a0.max() <= GRP and bin_b0.max() <= GRP
    nA = np.maximum(np.minimum(GRP, bin_tot - bin_b0), bin_tot - GRP)
    assert (nA >= bin_a0).all() and (bin_tot - nA <= GRP).all()

    bin_starts = np.zeros(NBINS + 1, np.int64)
    np.cumsum(bin_tot, out=bin_starts[1:])
    within = np.arange(E) - bin_starts[ebin_s]
    in_A = within < nA[ebin_s]
    gslot = np.where(in_A, within, within - nA[ebin_s])

    rowA = np.zeros((NBINS, GRP), np.int64)
    relA = np.full((NBINS, GRP), -1, np.int64)
    rowB = np.full((NBINS, GRP), B_BASE, np.int64)
    relB = np.full((NBINS, GRP), -1, np.int64)
    rowA[ebin_s[in_A], gslot[in_A]] = erow_s[in_A]
    relA[ebin_s[in_A], gslot[in_A]] = erel_s[in_A]
    rowB[ebin_s[~in_A], gslot[~in_A]] = erow_s[~in_A]
    relB[ebin_s[~in_A], gslot[~in_A]] = erel_s[~in_A]
    assert rowA.max() < A_LIMIT and rowB.min() >= B_BASE

    idxA_c = rowA.astype(np.int16).reshape(N_CORES, NWIN * GRP)
    idxB_c = (rowB - B_BASE).astype(np.int16).reshape(N_CORES, NWIN * GRP)

    def wrap(a):
        w16 = a.reshape(N_CORES, NWIN * GRP // 16, 16).transpose(0, 2, 1)
        return np.ascontiguousarray(np.tile(w16, (1, 8, 1)))

    idxA_w, idxB_w = wrap(idxA_c), wrap(idxB_c)

    S = np.zeros((N_CORES, P, CHUNKS, W), np.float16)

    def fill_S(rel, is_B):
        binid, slot = np.nonzero(rel >= 0)
        r = rel[binid, slot]
        core, win = binid // NWIN, binid % NWIN
        bank, w_local = win // BANK_W, win % BANK_W
        nwin_b = np.where(bank < NBANKS - 1, BANK_W, NWIN - BANK_W * (NBANKS - 1))
        sub = w_local * 2 + slot // P + (nwin_b * 2 if is_B else 0)
        S[core, slot % P, bank * 64 + sub, r] = np.float16(1.0)

    fill_S(relA, False)
    fill_S(relB, True)

    nodes_at = np.full((N_CORES, NODES), -1, np.int64)
    nodes_at[pos // NODES, pos % NODES] = np.arange(N)
    xT = np.zeros((N_CORES, P, KBLK, NODES), np.float32)
    for c in range(N_CORES):
        ids = nodes_at[c]
        valid = ids >= 0
        Xc = np.zeros((NODES, NFEAT), np.float32)
        Xc[valid] = x[ids[valid]]
        xT[c] = Xc.T.reshape(KBLK, P, NODES).transpose(1, 0, 2)

    W_in_r = np.ascontiguousarray(
        W_in.reshape(KBLK, P, HID).transpose(1, 0, 2)).astype(np.float32)
    b_in_rep = np.tile(b_in[None, :], (P, 4)).astype(np.float32)
    b_out_rep = np.tile(b_out[None, :], (P, 1)).astype(np.float32)
    Wc_hat = np.zeros((NUM_LAYERS, HID, HID), np.float16)
    diag = np.zeros((NUM_LAYERS, HID, HID), np.float32)
    for l in range(NUM_LAYERS):
        beta = math.log(THETA / (l + 1) + 1.0)
        Wt = (1.0 - beta) * np.eye(HID, dtype=np.float32) + beta * conv_W[l]
        Wc_hat[l] = (0.9 / G * Wt).astype(np.float16)
        diag[l] = np.eye(HID, dtype=np.float32) * (G ** (-float(l)))
    Wc_sb = np.ascontiguousarray(Wc_hat.transpose(1, 0, 2))
    diag_sb = np.ascontiguousarray(diag.transpose(1, 0, 2))

    per_core = []
    for c in range(N_CORES):
        per_core.append({
            "xT": xT[c], "idxA": idxA_w[c], "idxB": idxB_w[c], "S": S[c],
            "W_in": W_in_r, "b_in": b_in_rep, "Wc": Wc_sb, "diag": diag_sb,
            "W_out": W_out.astype(np.float16), "b_out": b_out_rep,
        })
    return per_core, nodes_at


# ---------------------------------------------------------------------------
# bass program
# ---------------------------------------------------------------------------

def _nwin_of_bank(b):
    return BANK_W if b < NBANKS - 1 else NWIN - BANK_W * (NBANKS - 1)


def build_program(repeat=1, skip_collective=False, skip_gather=False,
                  gather_ni=1024, queues=4):
    import concourse.bacc as bacc
    import concourse.tile as tile
    import concourse.mybir as mybir
    from concourse.masks import make_identity

    F32, F16, I16 = mybir.dt.float32, mybir.dt.float16, mybir.dt.int16
    nc = bacc.Bacc("TRN2", target_bir_lowering=False, debug=False,
                   num_devices=N_CORES, num_swdge_queues=queues)

    xT = nc.dram_tensor("xT", [P, KBLK, NODES], F32, kind="ExternalInput")
    idxA = nc.dram_tensor("idxA", [P, NWIN * GRP // 16], I16, kind="ExternalInput")
    idxB = nc.dram_tensor("idxB", [P, NWIN * GRP // 16], I16, kind="ExternalInput")
    S_in = nc.dram_tensor("S", [P, CHUNKS, W], F16, kind="ExternalInput")
    Wi_in = nc.dram_tensor("W_in", [P, KBLK, HID], F32, kind="ExternalInput")
    bi_in = nc.dram_tensor("b_in", [P, 4 * HID], F32, kind="ExternalInput")
    Wc_in = nc.dram_tensor("Wc", [HID, NUM_LAYERS, HID], F16, kind="ExternalInput")
    diag_in = nc.dram_tensor("diag", [HID, NUM_LAYERS, HID], F32, kind="ExternalInput")
    Wo_in = nc.dram_tensor("W_out", [HID, NCLASS], F16, kind="ExternalInput")
    bo_in = nc.dram_tensor("b_out", [P, NCLASS], F32, kind="ExternalInput")
    out_d = nc.dram_tensor("out", [NODES, NCLASS], F32, kind="ExternalOutput")

    with tile.TileContext(nc) as tc:
        with tc.tile_pool(name="dram", bufs=1, space="DRAM") as dram, \
             tc.tile_pool(name="res", bufs=1) as res, \
             tc.tile_pool(name="mpool", bufs=3) as mpool, \
             tc.tile_pool(name="psA", bufs=2, space="PSUM") as psA, \
             tc.tile_pool(name="psB", bufs=2, space="PSUM") as psB, \
             tc.tile_pool(name="psT", bufs=2, space="PSUM") as psT:

            table0 = dram.tile([NTAB, P], F16, name="table0")
            table1 = dram.tile([NTAB, P], F16, name="table1")
            shard_d = dram.tile([NODES, P], F16, name="shard")
            tables = [table0, table1]

            S_sb = res.tile([P, CHUNKS, W], F16)
            nc.sync.dma_start(S_sb[:], S_in[:])
            idxA_sb = res.tile([P, NWIN * GRP // 16], I16)
            nc.sync.dma_start(idxA_sb[:], idxA[:])
            idxB_sb = res.tile([P, NWIN * GRP // 16], I16)
            nc.sync.dma_start(idxB_sb[:], idxB[:])
            Wi_sb = res.tile([P, KBLK, HID], F32)
            nc.sync.dma_start(Wi_sb[:], Wi_in[:])
            bi_sb = res.tile([P, 4 * HID], F32)
            nc.sync.dma_start(bi_sb[:], bi_in[:])
            Wc_sb = res.tile([HID, NUM_LAYERS, HID], F16)
            nc.sync.dma_start(Wc_sb[:], Wc_in[:])
            diag_sb = res.tile([HID, NUM_LAYERS, HID], F32)
            nc.sync.dma_start(diag_sb[:], diag_in[:])
            Wo_sb = res.tile([HID, NCLASS], F16)
            nc.sync.dma_start(Wo_sb[:], Wo_in[:])
            bo_sb = res.tile([P, NCLASS], F32)
            nc.sync.dma_start(bo_sb[:], bo_in[:])
            ident = res.tile([P, P], F16)
            make_identity(nc, ident[:])

            hstage = res.tile([P, NBLK, P], F16)
            nc.vector.memset(hstage[:], 0.0)
            x0t = res.tile([HID, NODES], F32)
            hc = res.tile([HID, NODES], F16)
            h8T = res.tile([HID, NODES], F16)
            logits = res.tile([P, NBLK, NCLASS], F32)
            et = res.tile([P, NBLK, NCLASS], F32)
            mx = res.tile([P, NBLK], F32)
            sm = res.tile([P, NBLK], F32)
            lns = res.tile([P, NBLK], F32)

            for _rep in range(repeat):
                # -------- layer 0: h0 = relu(x @ W_in + b_in) --------
                for q in range(NBANKS):
                    nbl = 4 if q < NBANKS - 1 else 1
                    xt = mpool.tile([P, KBLK, 512], F32, tag="m", name="xt")
                    nc.sync.dma_start(xt[:, :, :nbl * P],
                                      xT[:, :, q * 512: q * 512 + nbl * P])
                    ps0 = psB.tile([P, 4 * HID], F32, tag="psB", name="ps0")
                    for t in range(nbl):
                        for k in range(KBLK):
                            nc.tensor.matmul(
                                ps0[:, t * HID:(t + 1) * HID],
                                lhsT=xt[:, k, t * P:(t + 1) * P],
                                rhs=Wi_sb[:, k, :],
                                start=(k == 0), stop=(k == KBLK - 1))
                    nc.vector.tensor_add(ps0[:, :nbl * HID], ps0[:, :nbl * HID],
                                         bi_sb[:, :nbl * HID])
                    nc.scalar.activation(
                        hstage[:, q * 4: q * 4 + nbl, :HID],
                        ps0[:, :nbl * HID],
                        mybir.ActivationFunctionType.Relu)
                for blk in range(NBLK):
                    pst = psT.tile([HID, P], F16, tag="psT", name="pst")
                    nc.tensor.transpose(pst[:], hstage[:, blk, :HID], ident[:])
                    nc.vector.tensor_scalar_mul(
                        x0t[:, blk * P:(blk + 1) * P], pst[:], 1.0 / 9.0)
                nc.sync.dma_start(
                    shard_d[:].rearrange("(b p) f -> p b f", p=P), hstage[:])
                if not skip_collective:
                    nc.gpsimd.collective_compute(
                        "AllGather", mybir.AluOpType.bypass,
                        replica_groups=[list(range(N_CORES))],
                        ins=[shard_d.opt()], outs=[tables[0].opt()])
                else:
                    for k in range(N_CORES):
                        nc.sync.dma_start(
                            tables[0][k * NODES:(k + 1) * NODES, :], shard_d[:])

                # -------- propagation layers -------------------------
                for layer in range(NUM_LAYERS):
                    tab = tables[layer % 2]
                    tab_next = tables[(layer + 1) % 2]
                    for b in range(NBANKS):
                        nw = _nwin_of_bank(b)
                        nchA = nw * 2
                        cols = nw * W
                        m = mpool.tile([P, 64, P], F16, tag="m", name="m")
                        if skip_gather:
                            # contiguous pseudo-gather: same bytes, sequential
                            nc.sync.dma_start(
                                m[:, :nchA * 2, :],
                                tab[:nchA * 2 * P, :]
                                .rearrange("(c p) f -> p c f", p=P))
                        else:
                            ni = min(gather_ni, nchA * P)
                            nsub = (nchA * P) // ni
                            nc_sub = ni // P          # chunks per sub-call
                            for j in range(nsub):
                                qn = (b * 2 * nsub + 2 * j) % queues
                                nc.gpsimd.dma_gather(
                                    out_ap=m[:, j * nc_sub:(j + 1) * nc_sub, :],
                                    in_ap=tab[:A_LIMIT, :],
                                    idxs_ap=idxA_sb[:, b * 256 + j * (ni // 16):
                                                    b * 256 + (j + 1) * (ni // 16)],
                                    num_idxs=ni, num_idxs_reg=ni,
                                    elem_size=P,
                                    single_packet=(ni <= 1024),
                                    queue_num=qn)
                                nc.gpsimd.dma_gather(
                                    out_ap=m[:, nchA + j * nc_sub:
                                             nchA + (j + 1) * nc_sub, :],
                                    in_ap=tab[B_BASE:, :],
                                    idxs_ap=idxB_sb[:, b * 256 + j * (ni // 16):
                                                    b * 256 + (j + 1) * (ni // 16)],
                                    num_idxs=ni, num_idxs_reg=ni,
                                    elem_size=P,
                                    single_packet=(ni <= 1024),
                                    queue_num=(qn + 1) % queues)
                        pA = psA.tile([P, BANK_W * W], F32, tag="psA", name="pA")
                        nc.tensor.matmul(
                            pA[:HID, :cols],
                            lhsT=diag_sb[:, layer, :],
                            rhs=x0t[:, b * 512: b * 512 + cols],
                            start=True, stop=False, skip_group_check=True)
                        for w in range(nw):
                            col = w * W
                            for t, sub in enumerate(
                                    (w * 2, w * 2 + 1,
                                     nchA + w * 2, nchA + w * 2 + 1)):
                                nc.tensor.matmul(
                                    pA[:HID, col:col + W],
                                    lhsT=m[:, sub, :HID],
                                    rhs=S_sb[:, b * 64 + sub, :],
                                    start=False,
                                    stop=(w == nw - 1 and t == 3),
                                    skip_group_check=True)
                        nc.vector.tensor_copy(
                            hc[:, b * 512: b * 512 + cols], pA[:HID, :cols])
                        if layer < NUM_LAYERS - 1:
                            ps1 = psB.tile([P, 4 * HID], F32, tag="psB", name="ps1")
                            nbl = cols // P
                            for t in range(nbl):
                                nc.tensor.matmul(
                                    ps1[:, t * HID:(t + 1) * HID],
                                    lhsT=hc[:, b * 512 + t * P:
                                            b * 512 + (t + 1) * P],
                                    rhs=Wc_sb[:, layer, :],
                                    start=True, stop=True)
                            nc.scalar.activation(
                                hstage[:, b * 4: b * 4 + nbl, :HID],
                                ps1[:, :nbl * HID],
                                mybir.ActivationFunctionType.Relu)
                        else:
                            pC = psA.tile([P, BANK_W * W], F32, tag="psA", name="pC")
                            nc.tensor.matmul(
                                pC[:HID, :cols],
                                lhsT=Wc_sb[:, layer, :],
                                rhs=hc[:, b * 512: b * 512 + cols],
                                start=True, stop=True)
                            nc.scalar.activation(
                                h8T[:, b * 512: b * 512 + cols],
                                pC[:HID, :cols],
                                mybir.ActivationFunctionType.Relu)
                    if layer < NUM_LAYERS - 1:
                        nc.sync.dma_start(
                            shard_d[:].rearrange("(b p) f -> p b f", p=P),
                            hstage[:])
                        if not skip_collective:
                            nc.gpsimd.collective_compute(
                                "AllGather", mybir.AluOpType.bypass,
                                replica_groups=[list(range(N_CORES))],
                                ins=[shard_d.opt()], outs=[tab_next.opt()])
                        else:
                            for k in range(N_CORES):
                                nc.sync.dma_start(
                                    tab_next[k * NODES:(k + 1) * NODES, :],
                                    shard_d[:])

                # -------- output head --------------------------------
                for blk in range(NBLK):
                    psD = psB.tile([P, 4 * HID], F32, tag="psB", name="psD")
                    nc.tensor.matmul(
                        psD[:, :NCLASS],
                        lhsT=h8T[:, blk * P:(blk + 1) * P],
                        rhs=Wo_sb[:],
                        start=True, stop=True)
                    nc.vector.tensor_scalar_mul(
                        logits[:, blk, :], psD[:, :NCLASS],
                        float(G ** NUM_LAYERS))
                nc.vector.tensor_add(
                    logits[:], logits[:],
                    bo_sb[:, None, :].to_broadcast([P, NBLK, NCLASS]))
                nc.vector.tensor_reduce(mx[:], logits[:],
                                        axis=mybir.AxisListType.X,
                                        op=mybir.AluOpType.max)
                nc.vector.tensor_sub(
                    logits[:], logits[:],
                    mx[:, :, None].to_broadcast([P, NBLK, NCLASS]))
                nc.scalar.activation(et[:], logits[:],
                                     mybir.ActivationFunctionType.Exp)
                nc.vector.tensor_reduce(sm[:], et[:],
                                        axis=mybir.AxisListType.X,
                                        op=mybir.AluOpType.add)
                nc.scalar.activation(lns[:], sm[:],
                                     mybir.ActivationFunctionType.Ln)
                nc.vector.tensor_sub(
                    logits[:], logits[:],
                    lns[:, :, None].to_broadcast([P, NBLK, NCLASS]))
                nc.sync.dma_start(
                    out_d[:].rearrange("(b p) f -> p b f", p=P), logits[:])

    nc.compile()
    return nc


# ---------------------------------------------------------------------------
# entry point
# ---------------------------------------------------------------------------

_CACHED_NC = None


def kernel(**inputs):
    global _CACHED_NC
    import time
    from concourse.bass_utils import run_bass_kernel_spmd

    per_core, nodes_at = _host_prep(inputs)
    if _CACHED_NC is None:
        _CACHED_NC = build_program(repeat=1)
    nc = _CACHED_NC
    res = None
    for attempt in range(3):
        try:
            res = run_bass_kernel_spmd(nc, per_core, core_ids=list(range(N_CORES)))
            break
        except Exception:
            if attempt == 2:
                raise
            time.sleep(90)   # axon terminal auto-recovers from NRT wedges
    out = np.zeros((N, NCLASS), np.float32)
    for c in range(N_CORES):
        ids = nodes_at[c]
        valid = ids >= 0
        out[ids[valid]] = res.results[c]["out"][valid]
    return out
